# revision 1
# baseline (speedup 1.0000x reference)
"""Binary-weight 3x3 conv (sign(weight) then conv2d, pad=1) on 8 TRN2 cores.

Data-parallel over batch: 32 images -> 4 per core; the small binarized
weight is replicated. Per core: implicit GEMM over the 9 filter taps
accumulated in fp32 PSUM; each output tile is [128 co, 8 rows x 56 cols].

x layout in SBUF (per image, per 128-channel half): W-padded rows with a
stride of 57 and shared zero columns:
  offset 0          : shared zero ("col -1" of virtual row -1)
  virtual row v in [-1, 56] at offset 1 + (v+1)*57, 56 data cols + 1 zero col
  rows -1 and 56 are all zero (H padding)
A tap (kh, kw) for output rows r0..r0+7 reads offset 1 + (r0+kh)*WS + (kw-1)
with free dims [[57, 8], [1, 56]]; every matmul is a uniform [128x128]@[128x448].

MODE "bf16" (default): x loaded fp32 (HWDGE, ACT ring) into staging, DVE
casts into the padded bf16 layout; matmuls run at 1 col/cycle. ("f32r"
kept for reference; walrus rejects un-rounded fp32r producers.)

Startup optimizations: weights live in 4 per-(ih, oh) tiles so compute can
start after the two oh=0 slices land; image 0 is loaded in three row
chunks so the first block's data (rows 0..8) lands ASAP; 45 dummy matmuls
warm the PE HAM clock gate (1.2 -> 2.4 GHz) during the first DMA wait.
H-edge trim: the kh tap row falling entirely on zero padding is skipped
at image top/bottom blocks (the kh=1 tap goes first so the start=True
matmul still covers the whole psum tile).
"""

import numpy as np
import ml_dtypes

import concourse.bacc as bacc
import concourse.mybir as mybir
import concourse.tile as tile
from concourse.bass_utils import run_bass_kernel_spmd

MODE = "bf16"  # "bf16" | "f32r"

N_CORES = 8
B = 32
BPC = B // N_CORES  # images per core
C = 256
H = W = 56
HW = H * W  # 3136
WS = 57  # padded row stride
XPAD = 1 + 58 * WS + 1  # 3308 (trailing elem so 8*57 row-view slices stay in bounds)
RB = 8  # output rows per block
NBLK = H // RB  # 7
NF = RB * W  # 448 matmul free size
NTAP = 9
FOURS = ((0,), (1,), (2,), (3,), (4,), (5,), (6,))  # per-block PSUM groups
CHUNKS = ((0, 9), (9, 16), (25, 16), (41, 15))  # image-0 chunks: (start_row, n_rows)

_CACHE = {}


def _build_module(mode):
    wdt = mybir.dt.bfloat16 if mode == "bf16" else mybir.dt.float32
    nc = bacc.Bacc("TRN2", target_bir_lowering=False, debug=False, num_devices=N_CORES)
    x = nc.declare_dram_parameter("x", [BPC, C, H, W], mybir.dt.float32, isOutput=False)
    wt = nc.declare_dram_parameter("wt", [128, 2 * 2 * NTAP * 128], wdt, isOutput=False)
    out = nc.declare_dram_parameter("out", [BPC, C, H, W], mybir.dt.float32, isOutput=True)

    xf = x.ap().rearrange("b c h w -> b c (h w)")  # [4, 256, 3136]
    of = out.ap().rearrange("b c h w -> b c (h w)")

    with tile.TileContext(nc) as tc:
        with (
            tc.tile_pool(name="xpads", bufs=BPC * 2) as xpool,
            tc.tile_pool(name="xstg", bufs=3) as spool,
            tc.tile_pool(name="wts", bufs=4) as wpool,
            tc.tile_pool(name="osb", bufs=2) as opool,
            tc.tile_pool(name="psum", bufs=8, space="PSUM") as ppool,
        ):
            xpads = {}
            wts = {}

            def init_xpad(n, ih):
                t = xpool.tile([128, XPAD], wdt if mode == "bf16" else mybir.dt.float32,
                               tag="xpad", name=f"xpad_{n}_{ih}")
                xpads[(n, ih)] = t
                nc.vector.memset(t[:, 0 : 1 + WS], 0.0)
                nc.vector.memset(t[:, 1 + 57 * WS : XPAD], 0.0)
                trail = t[:, 2 * WS : 2 * WS + 56 * WS].rearrange(
                    "p (h w) -> p h w", w=WS
                )[:, :, 0:1]
                nc.vector.memset(trail, 0.0)

            # persistent staging tiles reused round-robin (fewer logical
            # tiles -> fewer Tile release EVSEMs in the exit barrier)
            stgs = [
                spool.tile([128, HW], mybir.dt.float32, tag="xstg",
                           name=f"xstg_{i}")
                for i in range(3)
            ]
            stg_idx = [0]

            def load_chunk(n, ih, r0, nr, eng=None):
                eng = eng or nc.scalar
                t = xpads[(n, ih)]
                src = xf[n, ih * 128 : (ih + 1) * 128, r0 * W : (r0 + nr) * W]
                dst = t[:, WS + 1 + r0 * WS : WS + 1 + (r0 + nr) * WS].rearrange(
                    "p (h w) -> p h w", w=WS
                )[:, :, 0:W]
                if mode == "bf16":
                    stg = stgs[stg_idx[0] % 3]
                    stg_idx[0] += 1
                    eng.dma_start(stg[:, : nr * W], src)
                    nc.vector.tensor_copy(dst, stg[:, : nr * W])
                else:
                    eng.dma_start(dst, src)

            def load_weight(ih, oh):
                wtile = wpool.tile([128, NTAP * 128], wdt, tag="wt",
                                   name=f"wt_{ih}_{oh}")
                wts[(ih, oh)] = wtile
                c0 = (ih * 2 + oh) * NTAP * 128
                nc.sync.dma_start(wtile[:], wt.ap()[:, c0 : c0 + NTAP * 128])

            def mm_ap(ap):
                return ap.bitcast(mybir.dt.float32r) if mode == "f32r" else ap

            # PE warmup: dummy matmuls on scratch data while the first input
            # chunk is in flight, so the HAM clock gate releases (1.2->2.4GHz)
            # before the real matmul stream starts (~3.4us of sustained PE
            # activity required).
            warm_sb = wpool.tile([128, 128], wdt, tag="warm_sb")
            nc.vector.memset(warm_sb[:], 0.0)
            # 8 persistent psum tiles (one per bank) reused round-robin; 4
            # persistent output-staging tiles. Tile's byte-range deps still
            # serialize reuse (start=True matmul waits the old drain), but
            # far fewer logical tiles means a much shorter release storm in
            # the exit barrier.
            psts = [
                ppool.tile([128, NF], mybir.dt.float32, tag="ps", name=f"ps_{i}")
                for i in range(8)
            ]
            # two whole-(n,oh) output staging tiles; blocks drain into them
            # and each (n,oh) stores with just 2 DMAs (blocks 0-5, block 6)
            osbs = [
                opool.tile([128, HW], mybir.dt.float32, tag="osb", name=f"osb_{i}")
                for i in range(2)
            ]
            warm_ps = psts[7]
            for _ in range(42):
                nc.tensor.matmul(warm_ps[:, 0:128], lhsT=mm_ap(warm_sb[:]),
                                 rhs=mm_ap(warm_sb[:]), start=True, stop=True)

            # critical path first: image-0 chunk 0 of both halves, then the
            # oh=0 weight slices, then the rest
            for ih in range(2):
                init_xpad(0, ih)
            # chunk 0 of the two halves on different HWDGE rings so both
            # dispatch immediately; the oh=0 weights follow on the sync ring
            load_chunk(0, 0, *CHUNKS[0], eng=nc.scalar)
            load_chunk(0, 1, *CHUNKS[0], eng=nc.sync)
            load_weight(0, 0)
            load_weight(1, 0)
            for r0, nr in CHUNKS[1:]:
                for ih in range(2):
                    load_chunk(0, ih, r0, nr)
            load_weight(0, 1)
            load_weight(1, 1)
            # spread the remaining image loads across both HWDGE rings
            for n in range(1, BPC):
                for ih in range(2):
                    init_xpad(n, ih)
                    load_chunk(n, ih, 0, H,
                               eng=nc.scalar if ih == 0 else nc.sync)

            gidx = [0]
            for n in range(BPC):
                for oh in range(2):
                    for blks in FOURS:
                        pss = [psts[(gidx[0] + j) % 8] for j in range(len(blks))]
                        gidx[0] += len(blks)
                        k = 0
                        for ih in range(2):
                            # center tap first so the start=True matmul covers
                            # the whole psum tile (trimmed taps only accumulate)
                            for kh in (1, 0, 2):
                                for kw in range(3):
                                    lhsT = mm_ap(
                                        wts[(ih, oh)][:, (kh * 3 + kw) * 128 :
                                                      (kh * 3 + kw + 1) * 128]
                                    )
                                    for j, blk in enumerate(blks):
                                        # H-edge trim: the tap row that falls
                                        # entirely on zero padding is skipped
                                        # (psum stays contiguous)
                                        rs, nr = blk * RB, RB
                                        if blk == 0 and kh == 0:
                                            rs, nr = 1, RB - 1
                                        elif blk == NBLK - 1 and kh == 2:
                                            nr = RB - 1
                                        off = 1 + (rs + kh) * WS + (kw - 1)
                                        rhs = xpads[(n, ih)][
                                            :, off : off + nr * WS
                                        ].rearrange("p (h w) -> p h w", w=WS)[:, :, 0:W]
                                        p0 = (rs - blk * RB) * W
                                        nc.tensor.matmul(
                                            pss[j][:, p0 : p0 + nr * W],
                                            lhsT=lhsT,
                                            rhs=mm_ap(rhs),
                                            start=(k == 0),
                                            stop=(k == 17),
                                        )
                                    k += 1
                        osb = osbs[(n * 2 + oh) % 2]
                        for j, blk in enumerate(blks):
                            nc.scalar.copy(
                                osb[:, blk * NF : (blk + 1) * NF], pss[j][:]
                            )
                            if blk == NBLK - 2:
                                nc.sync.dma_start(
                                    of[n, oh * 128 : (oh + 1) * 128, 0 : 6 * NF],
                                    osb[:, 0 : 6 * NF],
                                )
                            elif blk == NBLK - 1:
                                nc.sync.dma_start(
                                    of[n, oh * 128 : (oh + 1) * 128, 6 * NF : HW],
                                    osb[:, 6 * NF : HW],
                                )

    nc.compile()
    return nc


def _pack_weights(weight: np.ndarray, mode) -> np.ndarray:
    # lhsT tile for (ih, oh, kh, kw): [ci, co] = sign(w)[oh*128+co, ih*128+ci, kh, kw]
    bw = np.sign(weight.astype(np.float32))
    bw = bw.reshape(2, 128, 2, 128, 3, 3)  # [oh, co, ih, ci, kh, kw]
    bw = bw.transpose(3, 2, 0, 4, 5, 1)  # [ci, ih, oh, kh, kw, co]
    bw = np.ascontiguousarray(bw.reshape(128, 2 * 2 * NTAP * 128))
    return bw.astype(ml_dtypes.bfloat16) if mode == "bf16" else bw


def _get_nc():
    key = ("nc", MODE)
    if key not in _CACHE:
        _CACHE[key] = _build_module(MODE)
    return _CACHE[key]


def _run(x: np.ndarray, weight: np.ndarray, **spmd_kwargs):
    nc = _get_nc()
    wt = _pack_weights(weight, MODE)
    x = np.ascontiguousarray(x.astype(np.float32, copy=False))
    in_maps = [
        {"x": x[i * BPC : (i + 1) * BPC], "wt": wt} for i in range(N_CORES)
    ]
    res = run_bass_kernel_spmd(nc, in_maps, list(range(N_CORES)), **spmd_kwargs)
    out = np.concatenate([r["out"] for r in res.results], axis=0)
    return out, res


def kernel(x: np.ndarray, weight: np.ndarray) -> np.ndarray:
    out, _ = _run(x, weight)
    return out



# revision 3
# speedup vs baseline: 1.1807x; 1.1807x over previous
"""Binary-weight 3x3 conv (sign(weight) then conv2d, pad=1) on 8 TRN2 cores.

v2: F(2,3) Winograd along H, direct 3-tap conv along W, fp16 datapath.

Data-parallel over batch: 32 images -> 4 per core; weights replicated.
Per core the conv is decomposed as:
  - input transform along H (DVE): V[i][ci, ty, w] = BT(F2,3) combos of
    x rows 2ty-1..2ty+2 (4 plain tensor_tensor adds per (img, ih-half),
    all +-1 coefficients, fp16 2x perf mode).
  - main matmul (PE): M[i][co, ty, w] = sum_{ci, kw} U[i][co, ci, kw] *
    V[i][ci, ty, w+kw-1], with U = G(F2,3) @ sign(w) along kh (entries in
    {0, +-0.5, +-1, +-1.5}: exact in fp16, packed on host). MAC count is
    2/3 of direct conv: per (img, oh, 7-ty chunk) 24 matmuls of
    [128x128]@[128x392] instead of direct conv's 36.
  - drains (ACT): plain PSUM->SBUF fp16 copies.
  - output transform (DVE): Y[2ty] = m0+m1+m2, Y[2ty+1] = m1-m2-m3
    (4 tensor_tensor ops per (img, oh) over all 28 ty), written
    row-interleaved to fp16 staging; DMA'd out fp16, widened on host.

x is uploaded as fp16 (host cast, like the host-packed weights).
Numerics (numpy bit-accurate sim): rel err 5.4e-4 vs fp32 reference.

Layouts per (img, ih):
  xpad [128, 3304]: H-padded rows r in [-1,56] at offset (r+1)*56 (rows -1
    and 56 zeroed once; 56 trailing slack elems for AP slicing).
  V [128, 4*28*60]: plane i, tile row ty at (i*28+ty)*60; index 2+w holds
    col w in [0,56); indices 1 and 58 are zero W-padding (memset once;
    tap kw reads cols kw+1 .. kw+57 of each row).
"""

import numpy as np

import concourse.bacc as bacc
import concourse.mybir as mybir
import concourse.tile as tile
from concourse.bass_utils import run_bass_kernel_spmd

MODE = "f23h"  # kept for test.py compat

N_CORES = 8
B = 32
BPC = B // N_CORES  # images per core
C = 256
H = W = 56
HW = H * W  # 3136
TY = 28          # H tiles (2 output rows each)
NPL = 4          # transformed planes
VROW = 60        # V row width (2 pad + 56 + 2 pad)
XSZ = 58 * W + W          # 3304: 58 rows + slack for AP slicing
VSZ = NPL * TY * VROW     # 6720
OSZ = HW + W              # 3192: osb + slack for strided row writes
TYC = 7          # ty per chunk
NCH = TY // TYC  # 4 chunks per (img, oh)
NF = TYC * W     # 392 matmul free size
MW = NPL * 3 * 2 * 128    # weight cols per oh half
NWARM = 60

_CACHE = {}


def _build_module():
    fp16 = mybir.dt.float16
    nc = bacc.Bacc("TRN2", target_bir_lowering=False, debug=False, num_devices=N_CORES)
    x = nc.declare_dram_parameter("x", [BPC, C, HW], fp16, isOutput=False)
    wt = nc.declare_dram_parameter("wt", [2, 128, MW], fp16, isOutput=False)
    out = nc.declare_dram_parameter("out", [BPC, C, HW], fp16, isOutput=True)

    with tile.TileContext(nc) as tc:
        with (
            tc.tile_pool(name="xpads", bufs=4) as xpool,
            tc.tile_pool(name="vts", bufs=4) as vpool,
            tc.tile_pool(name="wts", bufs=2) as wpool,
            tc.tile_pool(name="msbs", bufs=3) as mpool,
            tc.tile_pool(name="osbs", bufs=3) as opool,
            tc.tile_pool(name="oscr", bufs=4) as spool,
            tc.tile_pool(name="psum", bufs=8, space="PSUM") as ppool,
        ):
            xpads = [xpool.tile([128, XSZ], fp16, tag="xpad", name=f"xpad_{j}")
                     for j in range(4)]
            vts = [vpool.tile([128, VSZ], fp16, tag="vt", name=f"vt_{j}")
                   for j in range(4)]
            wts = [wpool.tile([128, MW], fp16, tag="wt", name=f"wt_{oh}")
                   for oh in range(2)]
            msbs = [mpool.tile([128, NPL * TY * W], fp16, tag="msb", name=f"msb_{j}")
                    for j in range(3)]
            osbs = [opool.tile([128, OSZ], fp16, tag="osb", name=f"osb_{j}")
                    for j in range(3)]
            oscr = [spool.tile([128, TY * W], fp16, tag="oscr", name=f"oscr_{j}")
                    for j in range(4)]
            psts = [ppool.tile([128, NF], mybir.dt.float32, tag="ps", name=f"ps_{j}")
                    for j in range(8)]
            warm_sb = wpool.tile([128, 128], fp16, tag="warm")

            # one-time zeroing: V pad columns, xpad H-pad rows, warm tile
            nc.vector.memset(warm_sb[:], 0.0)
            for j in range(4):
                nc.vector.memset(vts[j][:], 0.0)
                nc.vector.memset(xpads[j][:, 0:W], 0.0)           # row -1
                nc.vector.memset(xpads[j][:, 57 * W : XSZ], 0.0)  # row 56 + slack

            def load_x(n, ih):
                t = xpads[(n * 2 + ih) % 4]
                eng = nc.scalar if ih == 0 else nc.sync
                eng.dma_start(t[:, W : W + HW], x.ap()[n, ih * 128 : (ih + 1) * 128, :])

            # critical path: image 0 x, then weights, then image 1
            load_x(0, 0)
            load_x(0, 1)
            nc.scalar.dma_start(wts[0][:], wt.ap()[0])
            nc.sync.dma_start(wts[1][:], wt.ap()[1])
            load_x(1, 0)
            load_x(1, 1)

            # PE warmup while DMAs land (HAM clock gate 1.2->2.4 GHz)
            warm_ps = psts[7]
            for _ in range(NWARM):
                nc.tensor.matmul(warm_ps[:, 0:128], lhsT=warm_sb[:], rhs=warm_sb[:],
                                 start=True, stop=True)

            def drows(xp, h, ty0, nty):
                # x rows (2*ty + h - 1) for ty in [ty0, ty0+nty): AP [nty, 56]
                off = (2 * ty0 + h) * W
                return xp[:, off : off + nty * 2 * W].rearrange(
                    "p (t w) -> p t w", w=2 * W)[:, :, 0:W]

            def vrows(vt, i, ty0, nty):
                off = (i * TY + ty0) * VROW
                return vt[:, off : off + nty * VROW].rearrange(
                    "p (t w) -> p t w", w=VROW)[:, :, 2 : 2 + W]

            def emit_v(n, ih, ty0, nty):
                xp = xpads[(n * 2 + ih) % 4]
                vt = vts[(n * 2 + ih) % 4]
                d = [drows(xp, h, ty0, nty) for h in range(4)]
                nc.vector.tensor_sub(vrows(vt, 0, ty0, nty), d[0], d[2])
                nc.vector.tensor_add(vrows(vt, 1, ty0, nty), d[1], d[2])
                nc.vector.tensor_sub(vrows(vt, 2, ty0, nty), d[2], d[1])
                nc.vector.tensor_sub(vrows(vt, 3, ty0, nty), d[1], d[3])

            def mm_rhs(n, ihf, i, ch, kw):
                vt = vts[(n * 2 + ihf) % 4]
                off = (i * TY + ch * TYC) * VROW
                return vt[:, off : off + TYC * VROW].rearrange(
                    "p (t w) -> p t w", w=VROW)[:, :, kw + 1 : kw + 1 + W]

            # startup: V for image 0 chunk 0 first, then the rest of image 0
            emit_v(0, 0, 0, TYC)
            emit_v(0, 1, 0, TYC)
            emit_v(0, 0, TYC, TY - TYC)
            emit_v(0, 1, TYC, TY - TYC)

            pp = 0
            for n in range(BPC):
                for oh in range(2):
                    msb = msbs[(n * 2 + oh) % 3]
                    osb = osbs[(n * 2 + oh) % 3]
                    for ch in range(NCH):
                        for i in range(NPL):
                            P = psts[pp % 8]
                            pp += 1
                            k = 0
                            for kw in range(3):
                                for ihf in range(2):
                                    c0 = ((i * 3 + kw) * 2 + ihf) * 128
                                    nc.tensor.matmul(
                                        P[:], lhsT=wts[oh][:, c0 : c0 + 128],
                                        rhs=mm_rhs(n, ihf, i, ch, kw),
                                        start=(k == 0), stop=(k == 5))
                                    k += 1
                            nc.scalar.copy(
                                msb[:, (i * NCH + ch) * NF : (i * NCH + ch + 1) * NF],
                                P[:])
                    # output transform over the whole (img, oh)
                    m = [msb[:, i * TY * W : (i + 1) * TY * W] for i in range(NPL)]
                    A = oscr[(n * 2 + oh) % 4 // 2 * 2]
                    Bs = oscr[(n * 2 + oh) % 4 // 2 * 2 + 1]

                    def yrows(s):
                        off = s * W
                        return osb[:, off : off + TY * 2 * W].rearrange(
                            "p (t w) -> p t w", w=2 * W)[:, :, 0:W]

                    nc.vector.tensor_add(A[:], m[0], m[1])
                    nc.vector.tensor_add(yrows(0), A[:], m[2])
                    nc.vector.tensor_sub(Bs[:], m[1], m[2])
                    nc.vector.tensor_sub(yrows(1), Bs[:], m[3])
                    nc.sync.dma_start(
                        out.ap()[n, oh * 128 : (oh + 1) * 128, :],
                        osb[:, 0:HW])
                    # prefetch next image's V (and x two images ahead)
                    if oh == 0 and n + 1 < BPC:
                        emit_v(n + 1, 0, 0, TY)
                        emit_v(n + 1, 1, 0, TY)
                        if n + 2 < BPC:
                            load_x(n + 2, 0)
                            load_x(n + 2, 1)

    nc.compile()
    return nc


def _pack_weights(weight: np.ndarray) -> np.ndarray:
    bw = np.sign(weight.astype(np.float32))  # [co 256, ci 256, kh 3, kw 3]
    G23 = np.array([[1, 0, 0], [0.5, 0.5, 0.5], [0.5, -0.5, 0.5], [0, 0, 1]],
                   dtype=np.float32)
    U = np.einsum("ik,ockw->iocw", G23, bw)  # [4, co, ci, kw]
    U = U.reshape(NPL, 2, 128, 2, 128, 3)    # [i, ohh, co, ihh, ci, kw]
    U = U.transpose(1, 4, 0, 5, 3, 2)        # [oh, ci, i, kw, ihf, co]
    return np.ascontiguousarray(U.reshape(2, 128, MW)).astype(np.float16)


def _get_nc():
    if "nc" not in _CACHE:
        _CACHE["nc"] = _build_module()
    return _CACHE["nc"]


def _run(x: np.ndarray, weight: np.ndarray, **spmd_kwargs):
    nc = _get_nc()
    wtp = _pack_weights(weight)
    xh = np.ascontiguousarray(x.astype(np.float16).reshape(B, C, HW))
    in_maps = [
        {"x": xh[i * BPC : (i + 1) * BPC], "wt": wtp} for i in range(N_CORES)
    ]
    res = run_bass_kernel_spmd(nc, in_maps, list(range(N_CORES)), **spmd_kwargs)
    out = np.concatenate([r["out"] for r in res.results], axis=0)
    out = out.astype(np.float32).reshape(B, C, H, W)
    return out, res


def kernel(x: np.ndarray, weight: np.ndarray) -> np.ndarray:
    out, _ = _run(x, weight)
    return out


# revision 10
# speedup vs baseline: 1.3263x; 1.1233x over previous
"""Binary-weight 3x3 conv (sign(weight) then conv2d, pad=1) on 8 TRN2 cores.

v2: F(2,3) Winograd along H, direct 3-tap conv along W, fp16 datapath.

Data-parallel over batch: 32 images -> 4 per core; weights replicated.
Per core the conv is decomposed as:
  - input transform along H (DVE): V[i][ci, ty, w] = BT(F2,3) combos of
    x rows 2ty-1..2ty+2 (4 plain tensor_tensor adds per (img, ih-half),
    all +-1 coefficients, fp16 2x perf mode).
  - main matmul (PE): M[i][co, ty, w] = sum_{ci, kw} U[i][co, ci, kw] *
    V[i][ci, ty, w+kw-1], with U = G(F2,3) @ sign(w) along kh (entries in
    {0, +-0.5, +-1, +-1.5}: exact in fp16, packed on host). MAC count is
    2/3 of direct conv: per (img, oh, 7-ty chunk) 24 matmuls of
    [128x128]@[128x392] instead of direct conv's 36.
  - drains (ACT): plain PSUM->SBUF fp16 copies.
  - output transform (DVE): Y[2ty] = m0+m1+m2, Y[2ty+1] = m1-m2-m3
    (4 tensor_tensor ops per (img, oh) over all 28 ty), written
    row-interleaved to fp16 staging; DMA'd out fp16, widened on host.

x is uploaded as fp16 (host cast, like the host-packed weights).
Numerics (numpy bit-accurate sim): rel err 5.4e-4 vs fp32 reference.

Layouts per (img, ih):
  xpad [128, 3304]: H-padded rows r in [-1,56] at offset (r+1)*56 (rows -1
    and 56 zeroed once; 56 trailing slack elems for AP slicing).
  V [128, 4*28*60]: plane i, tile row ty at (i*28+ty)*60; index 2+w holds
    col w in [0,56); indices 1 and 58 are zero W-padding (memset once;
    tap kw reads cols kw+1 .. kw+57 of each row).
"""

import numpy as np

import concourse.bacc as bacc
import concourse.mybir as mybir
import concourse.tile as tile
from concourse.bass_utils import run_bass_kernel_spmd

MODE = "f23h"  # kept for test.py compat

N_CORES = 8
B = 32
BPC = B // N_CORES  # images per core
C = 256
H = W = 56
HW = H * W  # 3136
TY = 28          # H tiles (2 output rows each)
NPL = 4          # transformed planes
VROW = 60        # V row width (2 pad + 56 + 2 pad)
XSZ = 58 * W + W          # 3304: 58 rows + slack for AP slicing
VSZ = NPL * TY * VROW     # 6720
OSZ = HW + W              # 3192: osb + slack for strided row writes
TYC = 7          # ty per chunk
NCH = TY // TYC  # 4 chunks per (img, oh)
NF = TYC * W     # 392 matmul free size
MW = NPL * 3 * 2 * 128    # weight cols per oh half
NWARM = 40

_CACHE = {}


def _build_module():
    fp16 = mybir.dt.float16
    nc = bacc.Bacc("TRN2", target_bir_lowering=False, debug=False, num_devices=N_CORES)
    x = nc.declare_dram_parameter("x", [BPC, C, HW], fp16, isOutput=False)
    wt = nc.declare_dram_parameter("wt", [2, 128, MW], fp16, isOutput=False)
    out = nc.declare_dram_parameter("out", [BPC, C, HW], fp16, isOutput=True)

    with tile.TileContext(nc) as tc:
        with (
            tc.tile_pool(name="xpads", bufs=4) as xpool,
            tc.tile_pool(name="vts", bufs=4) as vpool,
            tc.tile_pool(name="wts", bufs=2) as wpool,
            tc.tile_pool(name="msbs", bufs=3) as mpool,
            tc.tile_pool(name="osbs", bufs=3) as opool,
            tc.tile_pool(name="oscr", bufs=4) as spool,
            tc.tile_pool(name="psum", bufs=8, space="PSUM") as ppool,
        ):
            xpads = [xpool.tile([128, XSZ], fp16, tag="xpad", name=f"xpad_{j}")
                     for j in range(4)]
            vts = [vpool.tile([128, VSZ], fp16, tag="vt", name=f"vt_{j}")
                   for j in range(4)]
            wts = [wpool.tile([128, MW], fp16, tag="wt", name=f"wt_{oh}")
                   for oh in range(2)]
            msbs = [mpool.tile([128, NPL * TY * W], fp16, tag="msb", name=f"msb_{j}")
                    for j in range(3)]
            osbs = [opool.tile([128, OSZ], fp16, tag="osb", name=f"osb_{j}")
                    for j in range(3)]
            oscr = [spool.tile([128, TY * W], fp16, tag="oscr", name=f"oscr_{j}")
                    for j in range(4)]
            psts = [ppool.tile([128, NF], mybir.dt.float32, tag="ps", name=f"ps_{j}")
                    for j in range(8)]
            warm_sb = wpool.tile([128, 128], fp16, tag="warm")

            # one-time zeroing: V pad columns (strided 2-col, cheap) and xpad
            # H-pad rows. Only image-0's buffers (0,1) upfront; buffers 2,3
            # are zeroed after image-0's transform is queued (deferred_zero).
            nc.vector.memset(warm_sb[:], 0.0)

            def zero_pads(j):
                vrr = vts[j][:].rearrange("p (t w) -> p t w", w=VROW)
                nc.vector.memset(vrr[:, :, 0:2], 0.0)
                nc.vector.memset(vrr[:, :, 58:60], 0.0)
                nc.vector.memset(xpads[j][:, 0:W], 0.0)           # row -1
                nc.vector.memset(xpads[j][:, 57 * W : XSZ], 0.0)  # row 56 + slack

            zero_pads(0)
            zero_pads(1)

            def load_x(n, ih):
                t = xpads[(n * 2 + ih) % 4]
                eng = nc.scalar if ih == 0 else nc.sync
                eng.dma_start(t[:, W : W + HW], x.ap()[n, ih * 128 : (ih + 1) * 128, :])

            # critical path: image 0 x, then weights, then image 1
            load_x(0, 0)
            load_x(0, 1)
            nc.scalar.dma_start(wts[0][:], wt.ap()[0])
            nc.sync.dma_start(wts[1][:], wt.ap()[1])
            load_x(1, 0)
            load_x(1, 1)

            # PE warmup while DMAs land (HAM clock gate 1.2->2.4 GHz)
            warm_ps = psts[7]
            for _ in range(NWARM):
                nc.tensor.matmul(warm_ps[:, 0:128], lhsT=warm_sb[:], rhs=warm_sb[:],
                                 start=True, stop=True)

            def drows(xp, h, ty0, nty):
                # x rows (2*ty + h - 1) for ty in [ty0, ty0+nty): AP [nty, 56]
                off = (2 * ty0 + h) * W
                return xp[:, off : off + nty * 2 * W].rearrange(
                    "p (t w) -> p t w", w=2 * W)[:, :, 0:W]

            def vrows(vt, i, ty0, nty):
                off = (i * TY + ty0) * VROW
                return vt[:, off : off + nty * VROW].rearrange(
                    "p (t w) -> p t w", w=VROW)[:, :, 2 : 2 + W]

            def emit_v(n, ih, ty0, nty):
                xp = xpads[(n * 2 + ih) % 4]
                vt = vts[(n * 2 + ih) % 4]
                d = [drows(xp, h, ty0, nty) for h in range(4)]
                nc.vector.tensor_sub(vrows(vt, 0, ty0, nty), d[0], d[2])
                nc.vector.tensor_add(vrows(vt, 1, ty0, nty), d[1], d[2])
                nc.vector.tensor_sub(vrows(vt, 2, ty0, nty), d[2], d[1])
                nc.vector.tensor_sub(vrows(vt, 3, ty0, nty), d[1], d[3])

            def mm_rhs(n, ihf, i, ch, kw):
                vt = vts[(n * 2 + ihf) % 4]
                off = (i * TY + ch * TYC) * VROW
                return vt[:, off : off + TYC * VROW].rearrange(
                    "p (t w) -> p t w", w=VROW)[:, :, kw + 1 : kw + 1 + W]

            # startup: V for image 0 chunk 0 first, then the rest of image 0
            emit_v(0, 0, 0, TYC)
            emit_v(0, 1, 0, TYC)
            emit_v(0, 0, TYC, TY - TYC)
            emit_v(0, 1, TYC, TY - TYC)
            zero_pads(2)
            zero_pads(3)

            def yrows(osb, s, ty0, nty):
                off = ty0 * 2 * W + s * W
                return osb[:, off : off + nty * 2 * W].rearrange(
                    "p (t w) -> p t w", w=2 * W)[:, :, 0:W]

            def emit_combos(n, oh, msb, osb, ty0, nty, sidx):
                # Y[2ty] = m0+m1+m2, Y[2ty+1] = m1-m2-m3 over ty-range
                # (msb chunk-major layout: ty-range == chunk-range * NF)
                m = [msb[:, i * TY * W + ty0 * W : i * TY * W + (ty0 + nty) * W]
                     for i in range(NPL)]
                A = oscr[sidx * 2][:, 0 : nty * W]
                Bs = oscr[sidx * 2 + 1][:, 0 : nty * W]
                nc.vector.tensor_add(A, m[0], m[1])
                nc.vector.tensor_add(yrows(osb, 0, ty0, nty), A, m[2])
                nc.vector.tensor_sub(Bs, m[1], m[2])
                nc.vector.tensor_sub(yrows(osb, 1, ty0, nty), Bs, m[3])

            pp = 0
            for n in range(BPC):
                for oh in range(2):
                    last = (n == BPC - 1 and oh == 1)
                    msb = msbs[(n * 2 + oh) % 3]
                    osb = osbs[(n * 2 + oh) % 3]
                    if not last:
                        for i in range(NPL):
                            # weight-reuse order: same lhsT serves all 4 chunks
                            k = 0
                            for kw in range(3):
                                for ihf in range(2):
                                    c0 = ((i * 3 + kw) * 2 + ihf) * 128
                                    for ch in range(NCH):
                                        nc.tensor.matmul(
                                            psts[(pp + ch) % 8][:],
                                            lhsT=wts[oh][:, c0 : c0 + 128],
                                            rhs=mm_rhs(n, ihf, i, ch, kw),
                                            start=(k == 0), stop=(k == 5))
                                    k += 1
                            for ch in range(NCH):
                                nc.scalar.copy(
                                    msb[:, (i * NCH + ch) * NF :
                                         (i * NCH + ch + 1) * NF],
                                    psts[(pp + ch) % 8][:])
                            pp += NCH
                        emit_combos(n, oh, msb, osb, 0, TY, (n * 2 + oh) % 2)
                        nc.sync.dma_start(
                            out.ap()[n, oh * 128 : (oh + 1) * 128, :],
                            osb[:, 0:HW])
                    else:
                        # last (img, oh): chunk-outer so drains/combos/DMA
                        # overlap the matmul stream and the tail stays short
                        for ch in range(NCH):
                            for i in range(NPL):
                                P = psts[pp % 8]
                                pp += 1
                                k = 0
                                for kw in range(3):
                                    for ihf in range(2):
                                        c0 = ((i * 3 + kw) * 2 + ihf) * 128
                                        nc.tensor.matmul(
                                            P[:], lhsT=wts[oh][:, c0 : c0 + 128],
                                            rhs=mm_rhs(n, ihf, i, ch, kw),
                                            start=(k == 0), stop=(k == 5))
                                        k += 1
                                nc.scalar.copy(
                                    msb[:, (i * NCH + ch) * NF :
                                         (i * NCH + ch + 1) * NF],
                                    P[:])
                            emit_combos(n, oh, msb, osb, ch * TYC, TYC, ch % 2)
                            if ch % 2 == 1:
                                nc.sync.dma_start(
                                    out.ap()[n, oh * 128 : (oh + 1) * 128,
                                             (ch - 1) * TYC * 2 * W :
                                             (ch + 1) * TYC * 2 * W],
                                    osb[:, (ch - 1) * TYC * 2 * W :
                                         (ch + 1) * TYC * 2 * W])
                    # prefetch next image's V (and x two images ahead)
                    if oh == 0 and n + 1 < BPC:
                        emit_v(n + 1, 0, 0, TY)
                        emit_v(n + 1, 1, 0, TY)
                        if n + 2 < BPC:
                            load_x(n + 2, 0)
                            load_x(n + 2, 1)

    nc.compile()
    return nc


def _pack_weights(weight: np.ndarray) -> np.ndarray:
    bw = np.sign(weight.astype(np.float32))  # [co 256, ci 256, kh 3, kw 3]
    G23 = np.array([[1, 0, 0], [0.5, 0.5, 0.5], [0.5, -0.5, 0.5], [0, 0, 1]],
                   dtype=np.float32)
    U = np.einsum("ik,ockw->iocw", G23, bw)  # [4, co, ci, kw]
    U = U.reshape(NPL, 2, 128, 2, 128, 3)    # [i, ohh, co, ihh, ci, kw]
    U = U.transpose(1, 4, 0, 5, 3, 2)        # [oh, ci, i, kw, ihf, co]
    return np.ascontiguousarray(U.reshape(2, 128, MW)).astype(np.float16)


def _get_nc():
    if "nc" not in _CACHE:
        _CACHE["nc"] = _build_module()
    return _CACHE["nc"]


def _run(x: np.ndarray, weight: np.ndarray, **spmd_kwargs):
    nc = _get_nc()
    wtp = _pack_weights(weight)
    xh = np.ascontiguousarray(x.astype(np.float16).reshape(B, C, HW))
    in_maps = [
        {"x": xh[i * BPC : (i + 1) * BPC], "wt": wtp} for i in range(N_CORES)
    ]
    res = run_bass_kernel_spmd(nc, in_maps, list(range(N_CORES)), **spmd_kwargs)
    out = np.concatenate([r["out"] for r in res.results], axis=0)
    out = out.astype(np.float32).reshape(B, C, H, W)
    return out, res


def kernel(x: np.ndarray, weight: np.ndarray) -> np.ndarray:
    out, _ = _run(x, weight)
    return out
